# revision 24
# baseline (speedup 1.0000x reference)
"""LIF recurrent-layer kernel for 8 TRN2 NeuronCores.

Strategy: data-parallel over batch (512 rows/core), full r/f/b on every core.
  u = relu(0.9*x + 0.1*(i_ + b))            (exact fp32, natural layout)
  r_out = u @ (signs[:,None] * |r|, diag=0) (fp16 matmuls, PSUM fp32 accum)
  f_out = u @ (signs[:,None] * |f|)
The EI signs are applied to u^T along the contraction dim (per-partition,
compile-time constants per k-tile), so the weight transform is just |.|;
the self-mask diagonal is zeroed with affine_select on the 16 tiles the
diagonal crosses.
"""
import sys

sys.path.insert(0, "/opt/trn_rl_repo")
import numpy as np

N_CORES = 8
B = 4096
N = 2048
OUT = 512
E = 1638
P = 128
BS = B // N_CORES  # 512 batch rows per core
MT = BS // P  # 4 m-tiles
KT = N // P  # 16 k-tiles
CH = 512
NCH = N // CH  # 4 r column chunks

_CACHE = {}


def _build():
    import concourse.bacc as bacc
    import concourse.mybir as mybir
    import concourse.tile as tile
    from concourse.masks import make_identity

    f32 = mybir.dt.float32
    f16 = mybir.dt.float16
    Act = mybir.ActivationFunctionType
    Alu = mybir.AluOpType

    nc = bacc.Bacc("TRN2", target_bir_lowering=False, debug=False, num_devices=N_CORES)
    i_d = nc.dram_tensor("i_s", [BS, N], f32, kind="ExternalInput")
    x_d = nc.dram_tensor("x_s", [BS, N], f32, kind="ExternalInput")
    r_d = nc.dram_tensor("r", [N, N], f32, kind="ExternalInput")
    f_d = nc.dram_tensor("f", [N, OUT], f32, kind="ExternalInput")
    b_d = nc.dram_tensor("b", [1, N], f32, kind="ExternalInput")
    u_d = nc.dram_tensor("u_s", [BS, N], f32, kind="ExternalOutput")
    ro_d = nc.dram_tensor("r_out_s", [BS, N], f32, kind="ExternalOutput")
    fo_d = nc.dram_tensor("f_out_s", [BS, OUT], f32, kind="ExternalOutput")

    with tile.TileContext(nc) as tc:
        with (
            tc.tile_pool(name="const", bufs=1) as const,
            tc.tile_pool(name="bpool", bufs=1) as bpool,
            tc.tile_pool(name="xi", bufs=3) as xi,
            tc.tile_pool(name="upool", bufs=2) as upool,
            tc.tile_pool(name="utp", bufs=1) as utp,
            tc.tile_pool(name="wp", bufs=6) as wp,
            tc.tile_pool(name="wep", bufs=3) as wep,
            tc.tile_pool(name="ost", bufs=4) as ost,
            tc.tile_pool(name="accp", bufs=6, space="PSUM") as accp,
            tc.tile_pool(name="tpsp", bufs=2, space="PSUM") as tpsp,
        ):
            identity = const.tile([P, P], f32)
            make_identity(nc, identity)

            # u^T slab: partition = k-within-tile, cols = (mi,ki,batch-within-tile)
            uT = utp.tile([P, MT * KT * P], f16)

            # ---- Phase B: u = relu(...), store u, build u^T with EI signs
            # x/i for mi=0 are issued before the 1-MiB b broadcast so the
            # elementwise chain starts as early as possible.
            xts, its = [], []
            for mi in range(MT):
                x_t = xi.tile([P, N], f32, name="x_t", bufs=3)
                i_t = xi.tile([P, N], f32, name="i_t", bufs=3)
                nc.sync.dma_start(x_t[:], x_d[mi * P : (mi + 1) * P, :])
                nc.sync.dma_start(i_t[:], i_d[mi * P : (mi + 1) * P, :])
                xts.append(x_t)
                its.append(i_t)
                if mi == 0:
                    # b broadcast across partitions via step-0 DMA; the 0.1 LIF
                    # coefficient folds into the relu scale (relu is positively
                    # homogeneous).
                    b01 = bpool.tile([P, N], f32)
                    nc.sync.dma_start(b01[:], b_d[:].partition_broadcast(P))
            for mi in range(MT):
                x_t, i_t = xts[mi], its[mi]
                u_nat = upool.tile([P, N], f32, name="u_nat")
                # s = 9*x + i ; s2 = s + b ; u = relu(0.1*s2). For mi=0 the
                # ops run in halves so the first transposes start sooner.
                halves = 2 if mi == 0 else 1
                H = N // halves
                for h in range(halves):
                    sl = slice(h * H, (h + 1) * H)
                    nc.vector.scalar_tensor_tensor(
                        i_t[:, sl], x_t[:, sl], 9.0, i_t[:, sl], Alu.mult, Alu.add
                    )
                    nc.vector.scalar_tensor_tensor(
                        x_t[:, sl], i_t[:, sl], 0.0, b01[:, sl], Alu.add, Alu.add
                    )
                    nc.scalar.activation(u_nat[:, sl], x_t[:, sl], Act.Relu, scale=0.1)
                nc.sync.dma_start(u_d[mi * P : (mi + 1) * P, :], u_nat[:])

                # transpose 4 k-tiles per psum bank, evict with sign fold
                for g in range(KT // 4):
                    tps = tpsp.tile([P, 4 * P], f32, name="tps")
                    for j in range(4):
                        ki = g * 4 + j
                        nc.tensor.transpose(
                            tps[:, j * P : (j + 1) * P],
                            u_nat[:, ki * P : (ki + 1) * P],
                            identity[:],
                        )
                    base = (mi * KT + g * 4) * P
                    dst = uT[:, base : base + 4 * P]
                    if g < 3:
                        nc.vector.tensor_copy(dst, tps[:])
                    else:
                        # k-tiles 12..15: inhibitory rows negate; then repair
                        # ki=12 partitions 0..101 (k<1638 are excitatory)
                        nc.vector.tensor_scalar_mul(dst, tps[:], -1.0)
                        nc.vector.tensor_copy(
                            uT[0 : E - 12 * P, base : base + P],
                            tps[0 : E - 12 * P, 0:P],
                        )

            # ---- Phase C: weights (abs + diag mask) and matmuls, chunk by
            # chunk: groups ordered (nj, mi) so consecutive groups reuse the
            # same 16 weight tiles while the next chunk's loads flow in.
            # (All-resident variants with deeper wep bufs measured 10-25% worse,
            # three times — keep the flowing-chunk structure.)
            _wpair = {}
            for nj in range(NCH + 1):  # 0..3 = r chunks, 4 = f
                wts = []
                for ki in range(KT):
                    if nj < NCH:
                        if nj % 2 == 0:
                            w_t = wp.tile([P, 2 * CH], f32, name="w_raw", bufs=6)
                            nc.sync.dma_start(
                                w_t[:],
                                r_d[ki * P : (ki + 1) * P, nj * CH : (nj + 2) * CH],
                            )
                            _wpair[ki] = w_t
                        w_t = _wpair[ki]
                        sub = w_t[:, (nj % 2) * CH : (nj % 2 + 1) * CH]
                        if ki // 4 == nj:
                            # zero the diagonal: iota = (ki*P+p) - (nj*CH+c)
                            nc.gpsimd.affine_select(
                                out=sub,
                                in_=sub,
                                compare_op=Alu.not_equal,
                                fill=0.0,
                                base=ki * P - nj * CH,
                                channel_multiplier=1,
                                pattern=[[-1, CH]],
                            )
                    else:
                        f_t = wp.tile([P, CH], f32, name="f_raw", bufs=4)
                        nc.sync.dma_start(f_t[:], f_d[ki * P : (ki + 1) * P, :])
                        sub = f_t[:]
                    weff = wep.tile([P, CH], f16, name=f"we{ki}", bufs=3)
                    if ki % 2 == 0:
                        nc.scalar.activation(weff[:], sub, Act.Abs)
                    else:
                        nc.vector.scalar_tensor_tensor(
                            weff[:], sub, -1.0, sub, Alu.mult, Alu.max
                        )
                    wts.append(weff[:])
                for mi in range(MT):
                    acc = accp.tile([P, CH], f32, name="acc", bufs=6)
                    for ki in range(KT):
                        col = (mi * KT + ki) * P
                        nc.tensor.matmul(
                            acc[:],
                            uT[:, col : col + P],
                            wts[ki],
                            start=(ki == 0),
                            stop=(ki == KT - 1),
                        )
                    o_t = ost.tile([P, CH], f32, name="o_t", bufs=4)
                    nc.vector.tensor_copy(o_t[:], acc[:])
                    if nj < NCH:
                        nc.sync.dma_start(
                            ro_d[mi * P : (mi + 1) * P, nj * CH : (nj + 1) * CH], o_t[:]
                        )
                    else:
                        nc.sync.dma_start(fo_d[mi * P : (mi + 1) * P, :], o_t[:])

    nc.compile()
    return nc


def _get_nc():
    if "nc" not in _CACHE:
        _CACHE["nc"] = _build()
    return _CACHE["nc"]


def kernel(i_, x, r, f, b, _trace=False, _trace_kwargs=None):
    from concourse.bass_utils import run_bass_kernel_spmd

    nc = _get_nc()
    i_ = np.ascontiguousarray(np.asarray(i_, dtype=np.float32))
    x = np.ascontiguousarray(np.asarray(x, dtype=np.float32))
    r = np.ascontiguousarray(np.asarray(r, dtype=np.float32))
    f = np.ascontiguousarray(np.asarray(f, dtype=np.float32))
    b = np.ascontiguousarray(np.asarray(b, dtype=np.float32)).reshape(1, N)

    in_maps = [
        {
            "i_s": i_[c * BS : (c + 1) * BS],
            "x_s": x[c * BS : (c + 1) * BS],
            "r": r,
            "f": f,
            "b": b,
        }
        for c in range(N_CORES)
    ]
    kw = {}
    if _trace:
        kw["trace"] = True
        kw.update(_trace_kwargs or {})
    res = run_bass_kernel_spmd(nc, in_maps, list(range(N_CORES)), **kw)
    u = np.concatenate([res.results[c]["u_s"] for c in range(N_CORES)], axis=0)
    r_out = np.concatenate([res.results[c]["r_out_s"] for c in range(N_CORES)], axis=0)
    f_out = np.concatenate([res.results[c]["f_out_s"] for c in range(N_CORES)], axis=0)
    if _trace:
        return (f_out, r_out, u), res
    return (f_out, r_out, u)


# revision 26
# speedup vs baseline: 1.0201x; 1.0201x over previous
"""LIF recurrent-layer kernel for 8 TRN2 NeuronCores.

Strategy: data-parallel over batch (512 rows/core), full r/f/b on every core.
  u = relu(0.9*x + 0.1*(i_ + b))            (exact fp32, natural layout)
  r_out = u @ (signs[:,None] * |r|, diag=0) (fp16 matmuls, PSUM fp32 accum)
  f_out = u @ (signs[:,None] * |f|)
The EI signs are applied to u^T along the contraction dim (per-partition,
compile-time constants per k-tile), so the weight transform is just |.|;
the self-mask diagonal is zeroed with affine_select on the 16 tiles the
diagonal crosses.
"""
import sys

sys.path.insert(0, "/opt/trn_rl_repo")
import numpy as np

N_CORES = 8
B = 4096
N = 2048
OUT = 512
E = 1638
P = 128
BS = B // N_CORES  # 512 batch rows per core
MT = BS // P  # 4 m-tiles
KT = N // P  # 16 k-tiles
CH = 512
NCH = N // CH  # 4 r column chunks

_CACHE = {}


def _build():
    import concourse.bacc as bacc
    import concourse.mybir as mybir
    import concourse.tile as tile
    from concourse.masks import make_identity

    f32 = mybir.dt.float32
    f16 = mybir.dt.float16
    Act = mybir.ActivationFunctionType
    Alu = mybir.AluOpType

    nc = bacc.Bacc("TRN2", target_bir_lowering=False, debug=False, num_devices=N_CORES)
    i_d = nc.dram_tensor("i_s", [BS, N], f32, kind="ExternalInput")
    x_d = nc.dram_tensor("x_s", [BS, N], f32, kind="ExternalInput")
    r_d = nc.dram_tensor("r", [N, N], f32, kind="ExternalInput")
    f_d = nc.dram_tensor("f", [N, OUT], f32, kind="ExternalInput")
    b_d = nc.dram_tensor("b", [1, N], f32, kind="ExternalInput")
    u_d = nc.dram_tensor("u_s", [BS, N], f32, kind="ExternalOutput")
    ro_d = nc.dram_tensor("r_out_s", [BS, N], f32, kind="ExternalOutput")
    fo_d = nc.dram_tensor("f_out_s", [BS, OUT], f32, kind="ExternalOutput")

    with tile.TileContext(nc) as tc:
        with (
            tc.tile_pool(name="const", bufs=1) as const,
            tc.tile_pool(name="bpool", bufs=1) as bpool,
            tc.tile_pool(name="xi", bufs=3) as xi,
            tc.tile_pool(name="upool", bufs=2) as upool,
            tc.tile_pool(name="utp", bufs=1) as utp,
            tc.tile_pool(name="wp", bufs=6) as wp,
            tc.tile_pool(name="wep", bufs=3) as wep,
            tc.tile_pool(name="ost", bufs=4) as ost,
            tc.tile_pool(name="accp", bufs=6, space="PSUM") as accp,
            tc.tile_pool(name="tpsp", bufs=2, space="PSUM") as tpsp,
        ):
            identity = const.tile([P, P], f32)
            make_identity(nc, identity)

            # u^T slab: partition = k-within-tile, cols = (mi,ki,batch-within-tile)
            uT = utp.tile([P, MT * KT * P], f16)

            # ---- Phase B: u = relu(...), store u, build u^T with EI signs
            # x/i for mi=0 are issued before the 1-MiB b broadcast so the
            # elementwise chain starts as early as possible.
            xts, its = [], []
            for mi in range(MT):
                x_t = xi.tile([P, N], f32, name="x_t", bufs=3)
                i_t = xi.tile([P, N], f32, name="i_t", bufs=3)
                nc.sync.dma_start(x_t[:], x_d[mi * P : (mi + 1) * P, :])
                nc.sync.dma_start(i_t[:], i_d[mi * P : (mi + 1) * P, :])
                xts.append(x_t)
                its.append(i_t)
                if mi == 0:
                    # b broadcast across partitions via step-0 DMA; the 0.1 LIF
                    # coefficient folds into the relu scale (relu is positively
                    # homogeneous).
                    b01 = bpool.tile([P, N], f32)
                    nc.sync.dma_start(b01[:], b_d[:].partition_broadcast(P))
            for mi in range(MT):
                x_t, i_t = xts[mi], its[mi]
                u_nat = upool.tile([P, N], f32, name="u_nat")
                # s = 9*x + i ; s2 = s + b ; u = relu(0.1*s2). For mi=0 the
                # ops run in halves so the first transposes start sooner.
                halves = 2 if mi == 0 else 1
                H = N // halves
                for h in range(halves):
                    sl = slice(h * H, (h + 1) * H)
                    nc.vector.scalar_tensor_tensor(
                        i_t[:, sl], x_t[:, sl], 9.0, i_t[:, sl], Alu.mult, Alu.add
                    )
                    nc.vector.scalar_tensor_tensor(
                        x_t[:, sl], i_t[:, sl], 0.0, b01[:, sl], Alu.add, Alu.add
                    )
                    nc.scalar.activation(u_nat[:, sl], x_t[:, sl], Act.Relu, scale=0.1)
                nc.sync.dma_start(u_d[mi * P : (mi + 1) * P, :], u_nat[:])

                # transpose 4 k-tiles per psum bank, evict with sign fold
                for g in range(KT // 4):
                    tps = tpsp.tile([P, 4 * P], f32, name="tps")
                    for j in range(4):
                        ki = g * 4 + j
                        nc.tensor.transpose(
                            tps[:, j * P : (j + 1) * P],
                            u_nat[:, ki * P : (ki + 1) * P],
                            identity[:],
                        )
                    base = (mi * KT + g * 4) * P
                    dst = uT[:, base : base + 4 * P]
                    if g < 3:
                        nc.vector.tensor_copy(dst, tps[:])
                    else:
                        # k-tiles 12..15: inhibitory rows negate; then repair
                        # ki=12 partitions 0..101 (k<1638 are excitatory)
                        nc.vector.tensor_scalar_mul(dst, tps[:], -1.0)
                        nc.vector.tensor_copy(
                            uT[0 : E - 12 * P, base : base + P],
                            tps[0 : E - 12 * P, 0:P],
                        )

            # ---- Phase C: weights (abs + diag mask) and matmuls, chunk by
            # chunk: groups ordered (nj, mi) so consecutive groups reuse the
            # same 16 weight tiles while the next chunk's loads flow in.
            # (All-resident variants with deeper wep bufs measured 10-25% worse,
            # three times — keep the flowing-chunk structure.)
            _wpair = {}
            for nj in range(NCH + 1):  # 0..3 = r chunks, 4 = f
                wts = []
                for ki in range(KT):
                    if nj < NCH:
                        if nj % 2 == 0:
                            w_t = wp.tile([P, 2 * CH], f32, name="w_raw", bufs=6)
                            nc.sync.dma_start(
                                w_t[:],
                                r_d[ki * P : (ki + 1) * P, nj * CH : (nj + 2) * CH],
                            )
                            _wpair[ki] = w_t
                        w_t = _wpair[ki]
                        sub = w_t[:, (nj % 2) * CH : (nj % 2 + 1) * CH]
                        if ki // 4 == nj:
                            # zero the diagonal: iota = (ki*P+p) - (nj*CH+c)
                            nc.gpsimd.affine_select(
                                out=sub,
                                in_=sub,
                                compare_op=Alu.not_equal,
                                fill=0.0,
                                base=ki * P - nj * CH,
                                channel_multiplier=1,
                                pattern=[[-1, CH]],
                            )
                    else:
                        f_t = wp.tile([P, CH], f32, name="f_raw", bufs=4)
                        nc.sync.dma_start(f_t[:], f_d[ki * P : (ki + 1) * P, :])
                        sub = f_t[:]
                    weff = wep.tile([P, CH], f16, name=f"we{ki}", bufs=3)
                    if ki % 2 == 0:
                        nc.scalar.activation(weff[:], sub, Act.Abs)
                    else:
                        nc.vector.scalar_tensor_tensor(
                            weff[:], sub, -1.0, sub, Alu.mult, Alu.max
                        )
                    wts.append(weff[:])

                def _group(nj_, mi, wts_):
                    acc = accp.tile([P, CH], f32, name="acc", bufs=6)
                    for ki in range(KT):
                        col = (mi * KT + ki) * P
                        nc.tensor.matmul(
                            acc[:],
                            uT[:, col : col + P],
                            wts_[ki],
                            start=(ki == 0),
                            stop=(ki == KT - 1),
                        )
                    o_t = ost.tile([P, CH], f32, name="o_t", bufs=4)
                    nc.vector.tensor_copy(o_t[:], acc[:])
                    if nj_ < NCH:
                        nc.sync.dma_start(
                            ro_d[mi * P : (mi + 1) * P, nj_ * CH : (nj_ + 1) * CH],
                            o_t[:],
                        )
                    else:
                        nc.sync.dma_start(fo_d[mi * P : (mi + 1) * P, :], o_t[:])

                if nj < NCH and nj % 2 == 0:
                    _pend = (nj, wts)  # defer: emit with the odd sibling
                elif nj < NCH:
                    for mi in range(MT):
                        _group(_pend[0], mi, _pend[1])
                        _group(nj, mi, wts)
                else:
                    for mi in range(MT):
                        _group(nj, mi, wts)

    nc.compile()
    return nc


def _get_nc():
    if "nc" not in _CACHE:
        _CACHE["nc"] = _build()
    return _CACHE["nc"]


def kernel(i_, x, r, f, b, _trace=False, _trace_kwargs=None):
    from concourse.bass_utils import run_bass_kernel_spmd

    nc = _get_nc()
    i_ = np.ascontiguousarray(np.asarray(i_, dtype=np.float32))
    x = np.ascontiguousarray(np.asarray(x, dtype=np.float32))
    r = np.ascontiguousarray(np.asarray(r, dtype=np.float32))
    f = np.ascontiguousarray(np.asarray(f, dtype=np.float32))
    b = np.ascontiguousarray(np.asarray(b, dtype=np.float32)).reshape(1, N)

    in_maps = [
        {
            "i_s": i_[c * BS : (c + 1) * BS],
            "x_s": x[c * BS : (c + 1) * BS],
            "r": r,
            "f": f,
            "b": b,
        }
        for c in range(N_CORES)
    ]
    kw = {}
    if _trace:
        kw["trace"] = True
        kw.update(_trace_kwargs or {})
    res = run_bass_kernel_spmd(nc, in_maps, list(range(N_CORES)), **kw)
    u = np.concatenate([res.results[c]["u_s"] for c in range(N_CORES)], axis=0)
    r_out = np.concatenate([res.results[c]["r_out_s"] for c in range(N_CORES)], axis=0)
    f_out = np.concatenate([res.results[c]["f_out_s"] for c in range(N_CORES)], axis=0)
    if _trace:
        return (f_out, r_out, u), res
    return (f_out, r_out, u)


# revision 28
# speedup vs baseline: 1.0981x; 1.0764x over previous
"""LIF recurrent-layer kernel for 8 TRN2 NeuronCores.

Strategy: data-parallel over batch (512 rows/core), full r/f/b on every core.
  u = relu(0.9*x + 0.1*(i_ + b))            (exact fp32, natural layout)
  r_out = u @ (signs[:,None] * |r|, diag=0) (fp16 matmuls, PSUM fp32 accum)
  f_out = u @ (signs[:,None] * |f|)
The EI signs are applied to u^T along the contraction dim (per-partition,
compile-time constants per k-tile), so the weight transform is just |.|;
the self-mask diagonal is zeroed with affine_select on the 16 tiles the
diagonal crosses.
"""
import sys

sys.path.insert(0, "/opt/trn_rl_repo")
import numpy as np

N_CORES = 8
B = 4096
N = 2048
OUT = 512
E = 1638
P = 128
BS = B // N_CORES  # 512 batch rows per core
MT = BS // P  # 4 m-tiles
KT = N // P  # 16 k-tiles
CH = 512
NCH = N // CH  # 4 r column chunks

_CACHE = {}


def _build():
    import concourse.bacc as bacc
    import concourse.mybir as mybir
    import concourse.tile as tile
    from concourse.masks import make_identity

    f32 = mybir.dt.float32
    f16 = mybir.dt.float16
    Act = mybir.ActivationFunctionType
    Alu = mybir.AluOpType

    nc = bacc.Bacc("TRN2", target_bir_lowering=False, debug=False, num_devices=N_CORES)
    i_d = nc.dram_tensor("i_s", [BS, N], f32, kind="ExternalInput")
    x_d = nc.dram_tensor("x_s", [BS, N], f32, kind="ExternalInput")
    r_d = nc.dram_tensor("r", [N, N], f32, kind="ExternalInput")
    f_d = nc.dram_tensor("f", [N, OUT], f32, kind="ExternalInput")
    b_d = nc.dram_tensor("b", [1, N], f32, kind="ExternalInput")
    u_d = nc.dram_tensor("u_s", [BS, N], f32, kind="ExternalOutput")
    ro_d = nc.dram_tensor("r_out_s", [BS, N], f32, kind="ExternalOutput")
    fo_d = nc.dram_tensor("f_out_s", [BS, OUT], f32, kind="ExternalOutput")

    with tile.TileContext(nc) as tc:
        with (
            tc.tile_pool(name="const", bufs=1) as const,
            tc.tile_pool(name="bpool", bufs=1) as bpool,
            tc.tile_pool(name="xi", bufs=3) as xi,
            tc.tile_pool(name="upool", bufs=2) as upool,
            tc.tile_pool(name="utp", bufs=1) as utp,
            tc.tile_pool(name="wp", bufs=6) as wp,
            tc.tile_pool(name="wep", bufs=3) as wep,
            tc.tile_pool(name="ost", bufs=4) as ost,
            tc.tile_pool(name="accp", bufs=6, space="PSUM") as accp,
            tc.tile_pool(name="tpsp", bufs=2, space="PSUM") as tpsp,
        ):
            identity = const.tile([P, P], f32)
            make_identity(nc, identity)

            # u^T slab: partition = k-within-tile, cols = (mi,ki,batch-within-tile)
            uT = utp.tile([P, MT * KT * P], f16)

            # ---- Phase B: u = relu(...), store u, build u^T with EI signs
            # x/i for mi=0 are issued before the 1-MiB b broadcast so the
            # elementwise chain starts as early as possible.
            xts, its = [], []
            for mi in range(MT):
                x_t = xi.tile([P, N], f32, name="x_t", bufs=3)
                i_t = xi.tile([P, N], f32, name="i_t", bufs=3)
                nc.sync.dma_start(x_t[:], x_d[mi * P : (mi + 1) * P, :])
                nc.sync.dma_start(i_t[:], i_d[mi * P : (mi + 1) * P, :])
                xts.append(x_t)
                its.append(i_t)
                if mi == 0:
                    # b broadcast across partitions via step-0 DMA; the 0.1 LIF
                    # coefficient folds into the relu scale (relu is positively
                    # homogeneous).
                    b01 = bpool.tile([P, N], f32)
                    nc.sync.dma_start(b01[:], b_d[:].partition_broadcast(P))
            for mi in range(MT):
                x_t, i_t = xts[mi], its[mi]
                u_nat = upool.tile([P, N], f32, name="u_nat")
                # s = 9*x + i ; s2 = s + b ; u = relu(0.1*s2). For mi=0 the
                # ops run in halves so the first transposes start sooner.
                halves = 2 if mi == 0 else 1
                H = N // halves
                for h in range(halves):
                    sl = slice(h * H, (h + 1) * H)
                    nc.vector.scalar_tensor_tensor(
                        i_t[:, sl], x_t[:, sl], 9.0, i_t[:, sl], Alu.mult, Alu.add
                    )
                    nc.vector.scalar_tensor_tensor(
                        x_t[:, sl], i_t[:, sl], 0.0, b01[:, sl], Alu.add, Alu.add
                    )
                    nc.scalar.activation(u_nat[:, sl], x_t[:, sl], Act.Relu, scale=0.1)
                nc.sync.dma_start(u_d[mi * P : (mi + 1) * P, :], u_nat[:])

                # transpose 4 k-tiles per psum bank, evict with sign fold
                for g in range(KT // 4):
                    tps = tpsp.tile([P, 4 * P], f32, name="tps")
                    for j in range(4):
                        ki = g * 4 + j
                        nc.tensor.transpose(
                            tps[:, j * P : (j + 1) * P],
                            u_nat[:, ki * P : (ki + 1) * P],
                            identity[:],
                        )
                    base = (mi * KT + g * 4) * P
                    dst = uT[:, base : base + 4 * P]
                    if g < 3:
                        nc.vector.tensor_copy(dst, tps[:])
                    else:
                        # k-tiles 12..15: inhibitory rows negate; then repair
                        # ki=12 partitions 0..101 (k<1638 are excitatory)
                        nc.vector.tensor_scalar_mul(dst, tps[:], -1.0)
                        nc.vector.tensor_copy(
                            uT[0 : E - 12 * P, base : base + P],
                            tps[0 : E - 12 * P, 0:P],
                        )

            # ---- Phase C: weights (abs + diag mask) and matmuls, chunk by
            # chunk: groups ordered (nj, mi) so consecutive groups reuse the
            # same 16 weight tiles while the next chunk's loads flow in.
            # (All-resident variants with deeper wep bufs measured 10-25% worse,
            # three times — keep the flowing-chunk structure.)
            _wpair = {}
            for nj in range(NCH + 1):  # 0..3 = r chunks, 4 = f
                wts = []
                for ki in range(KT):
                    if nj < NCH:
                        if nj % 2 == 0:
                            w_t = wp.tile([P, 2 * CH], f32, name="w_raw", bufs=6)
                            nc.sync.dma_start(
                                w_t[:],
                                r_d[ki * P : (ki + 1) * P, nj * CH : (nj + 2) * CH],
                            )
                            _wpair[ki] = w_t
                        w_t = _wpair[ki]
                        sub = w_t[:, (nj % 2) * CH : (nj % 2 + 1) * CH]
                        if ki // 4 == nj:
                            # zero the diagonal: iota = (ki*P+p) - (nj*CH+c)
                            nc.gpsimd.affine_select(
                                out=sub,
                                in_=sub,
                                compare_op=Alu.not_equal,
                                fill=0.0,
                                base=ki * P - nj * CH,
                                channel_multiplier=1,
                                pattern=[[-1, CH]],
                            )
                    else:
                        f_t = wp.tile([P, CH], f32, name="f_raw", bufs=4)
                        nc.sync.dma_start(f_t[:], f_d[ki * P : (ki + 1) * P, :])
                        sub = f_t[:]
                    weff = wep.tile([P, CH], f16, name=f"we{ki}", bufs=3)
                    if ki % 2 == 0:
                        nc.scalar.activation(weff[:], sub, Act.Abs)
                    else:
                        nc.vector.scalar_tensor_tensor(
                            weff[:], sub, -1.0, sub, Alu.mult, Alu.max
                        )
                    wts.append(weff[:])

                def _group(nj_, mi, wts_):
                    acc = accp.tile([P, CH], f32, name="acc", bufs=6)
                    for ki in range(KT):
                        col = (mi * KT + ki) * P
                        nc.tensor.matmul(
                            acc[:],
                            uT[:, col : col + P],
                            wts_[ki],
                            start=(ki == 0),
                            stop=(ki == KT - 1),
                        )
                    o_t = ost.tile([P, CH], f32, name="o_t", bufs=4)
                    nc.vector.tensor_copy(o_t[:], acc[:])
                    if nj_ < NCH:
                        nc.sync.dma_start(
                            ro_d[mi * P : (mi + 1) * P, nj_ * CH : (nj_ + 1) * CH],
                            o_t[:],
                        )
                    else:
                        nc.sync.dma_start(fo_d[mi * P : (mi + 1) * P, :], o_t[:])

                if nj < NCH and nj % 2 == 0:
                    _pend = (nj, wts)  # defer: emit with the odd sibling
                elif nj < NCH:
                    for mi in range(MT):
                        _group(_pend[0], mi, _pend[1])
                        _group(nj, mi, wts)
                else:
                    for mi in range(MT):
                        _group(nj, mi, wts)

    nc.compile()
    return nc


def _get_nc():
    if "nc" not in _CACHE:
        _CACHE["nc"] = _build()
    return _CACHE["nc"]


def kernel(i_, x, r, f, b, _trace=False, _trace_kwargs=None):
    from concourse.bass_utils import run_bass_kernel_spmd

    nc = _get_nc()
    i_ = np.ascontiguousarray(np.asarray(i_, dtype=np.float32))
    x = np.ascontiguousarray(np.asarray(x, dtype=np.float32))
    r = np.ascontiguousarray(np.asarray(r, dtype=np.float32))
    f = np.ascontiguousarray(np.asarray(f, dtype=np.float32))
    b = np.ascontiguousarray(np.asarray(b, dtype=np.float32)).reshape(1, N)

    in_maps = [
        {
            "i_s": i_[c * BS : (c + 1) * BS],
            "x_s": x[c * BS : (c + 1) * BS],
            "r": r,
            "f": f,
            "b": b,
        }
        for c in range(N_CORES)
    ]
    kw = {}
    if _trace:
        kw["trace"] = True
        kw.update(_trace_kwargs or {})
    res = run_bass_kernel_spmd(nc, in_maps, list(range(N_CORES)), **kw)
    u = np.concatenate([res.results[c]["u_s"] for c in range(N_CORES)], axis=0)
    r_out = np.concatenate([res.results[c]["r_out_s"] for c in range(N_CORES)], axis=0)
    f_out = np.concatenate([res.results[c]["f_out_s"] for c in range(N_CORES)], axis=0)
    if _trace:
        return (f_out, r_out, u), res
    return (f_out, r_out, u)
